# revision 1
# baseline (speedup 1.0000x reference)
"""Trainium2 Bass kernel for the pairwise-similarity histogram loss.

Reference computation:
  sim = x @ x.T  (rows L2-normalized), upper-tri pairs (i<j)
  soft (triangular) binning of similarities into 51 bins, separately for
  label-equal (pos) and label-unequal (neg) pairs; loss = sum(hist_neg * cumsum(hist_pos)).

Device algorithm (8 NeuronCores, SPMD, data-parallel over sim rows):
  Host sorts rows by label and hands each core a column-rotated copy of
  x_sorted.T so every core's own 128 rows sit at rotated columns 0..127 —
  all cores run an identical program.

  Per core:
    - PE: sim_shard = xT[:,0:128].T @ xT   -> PSUM [128, 1024]
    - masks from iota/label compares; s' = triu_mask * (1 + sim) in [0, 2]
    - histogramming uses the identity
        F[k] = sum_p clip((s'_p - k*bw)/bw, 0, 1) = (R[k] - R[k+1]) / bw,
        R[k] = sum_p relu(s'_p - k*bw)
      each R[k] is ONE fused instruction (relu + per-partition accumulate)
      on DVE (tensor_scalar sub/max + accum_out) or ACT (activation Relu +
      accum_out), split across engines.
    - pos pairs: after label-sorting they all live in a 64-wide diagonal
      band; the band is extracted via a skewed DRAM bounce and binned on a
      tiny [128, 63] tile.  neg = triu - pos.
  Host: f64 combine of per-partition partial sums -> final scalar loss.
"""

import numpy as np

NBINS = 51
BW = 2.0 / (NBINS - 1)
BS, D = 1024, 128
N_CORES = 8
SH = BS // N_CORES  # 128 rows per core

# ---------------- configuration ----------------
USE_BAND = False          # pos histogram via diagonal band (else dense pos passes)
KLO, KHI = 13, 38          # R[k] computed on device for k in [KLO, KHI]; outside: closed form
PASS_DT = "float16"       # dtype of the masked s' tiles the bin passes read
BANDW = 64                # band width (covers within-class pair distance <= 63)

_CACHE = {}


def _build_program():
    import concourse.bass as bass
    import concourse.bacc as bacc
    import concourse.tile as tile
    import concourse.mybir as mybir
    from concourse.ap import AP

    F32 = mybir.dt.float32
    PDT = getattr(mybir.dt, PASS_DT)
    Alu = mybir.AluOpType
    Act = mybir.ActivationFunctionType

    ks = list(range(KLO, KHI + 1))  # R[k] passes on device

    # pass plan: (family, k) -> engine + column index
    passes = []
    fams = ["tri", "pos"] if USE_BAND else ["pos", "neg"]
    for fam in fams:
        for k in ks:
            passes.append((fam, k))
    # weighted greedy assignment across DVE/ACT/GPSIMD by modeled pass cost
    est = {"D": 2300.0, "A": 1250.0, "G": 2200.0}  # head start = prep busy
    cost = {"D": 1147.0, "A": 1253.0, "G": 1e12}
    plan = {}
    counts = {"D": 0, "A": 0, "G": 0}
    for fam, k in passes:
        eng = min(est, key=lambda e: est[e] + cost[e])
        est[eng] += cost[eng]
        plan[(fam, k)] = (eng, None)
    # column indexing per engine
    for key in plan:
        eng, _ = plan[key]
        plan[key] = (eng, counts[eng])
        counts[eng] += 1
    nD, nA, nG = counts["D"], counts["A"], counts["G"]
    NCNT = 4  # cntpos, cntneg, Spos, Sneg
    NOUT = nD + nA + nG + NCNT

    nc = bacc.Bacc("TRN2", target_bir_lowering=False, debug=False,
                   num_devices=N_CORES)

    xTrL = nc.dram_tensor("xTrL", [D, 512], F32, kind="ExternalInput")
    xTrR = nc.dram_tensor("xTrR", [D, 512], F32, kind="ExternalInput")
    collab = nc.dram_tensor("collab", [1, BS], F32, kind="ExternalInput")
    collab_bf = nc.dram_tensor("collab_bf", [1, BS], mybir.dt.bfloat16,
                               kind="ExternalInput")
    rowlab = nc.dram_tensor("rowlab", [SH, 1], F32, kind="ExternalInput")
    wrapcut = nc.dram_tensor("wrapcut", [SH, 1], F32, kind="ExternalInput")
    nk = len(ks)
    cvec = nc.dram_tensor("cvec", [SH, nk], F32, kind="ExternalInput")
    acc_out = nc.dram_tensor("acc", [SH, NOUT], F32, kind="ExternalOutput")

    with tile.TileContext(nc) as tc:
        with tc.tile_pool(name="main", bufs=1) as pool, \
             tc.tile_pool(name="psum", bufs=1, space="PSUM") as psum:
            xL = pool.tile([D, 512], F32)
            xR = pool.tile([D, 512], F32)
            nc.sync.dma_start(xL[:], xTrL[:])
            nc.sync.dma_start(xR[:], xTrR[:])

            rowlab_sb = pool.tile([SH, 1], F32)
            nc.sync.dma_start(rowlab_sb[:], rowlab[:])
            wrapcut_sb = pool.tile([SH, 1], F32)
            nc.sync.dma_start(wrapcut_sb[:], wrapcut[:])
            cvec_sb = pool.tile([SH, nk], F32)
            nc.sync.dma_start(cvec_sb[:], cvec[:])
            collab_bf_sb = pool.tile([1, BS], mybir.dt.bfloat16)
            nc.sync.dma_start(collab_bf_sb[:], collab_bf[:])

            ones_bf = pool.tile([1, D], mybir.dt.bfloat16)
            nc.vector.memset(ones_bf[:], 1.0)

            # iota along free dim (0..1023), and local row index (0..127)
            iotaT = pool.tile([SH, BS], F32)
            nc.gpsimd.iota(iotaT[:], pattern=[[1, BS]], base=0,
                           channel_multiplier=0,
                           allow_small_or_imprecise_dtypes=True)
            rowloc = pool.tile([SH, 1], F32)
            nc.gpsimd.iota(rowloc[:], pattern=[[0, 1]], base=0,
                           channel_multiplier=1,
                           allow_small_or_imprecise_dtypes=True)

            # sim = xT[:, 0:128].T @ xT  -> PSUM
            simP = psum.tile([SH, BS], F32)
            nc.tensor.matmul(simP[:, 0:512], xL[:, 0:D], xL[:])
            nc.tensor.matmul(simP[:, 512:BS], xL[:, 0:D], xR[:])

            # label broadcast via K=1 bf16 matmul (labels 0..31 exact in bf16)
            labmatP = psum.tile([SH, BS], F32)
            nc.tensor.matmul(labmatP[:, 0:512], ones_bf[:], collab_bf_sb[:, 0:512])
            nc.tensor.matmul(labmatP[:, 512:BS], ones_bf[:], collab_bf_sb[:, 512:BS])

            # triu mask (in rotated coords): (t > r) & (t < wrapcut)
            gtmask = pool.tile([SH, BS], F32)
            nc.gpsimd.tensor_scalar(gtmask[:], iotaT[:], rowloc[:], None,
                                    op0=Alu.is_gt)
            trimask = pool.tile([SH, BS], F32)
            nc.vector.scalar_tensor_tensor(trimask[:], iotaT[:], wrapcut_sb[:],
                                           gtmask[:], op0=Alu.is_lt, op1=Alu.mult)

            # s' = 1 + sim (ACT, evacuates PSUM)
            splus = pool.tile([SH, BS], F32)
            nc.scalar.activation(splus[:], simP[:], Act.Identity, bias=1.0)

            cnts = pool.tile([SH, NCNT], F32)

            # pos/neg masks and masked s' tensors
            mpos = pool.tile([SH, BS], F32)
            nc.vector.scalar_tensor_tensor(mpos[:], labmatP[:], rowlab_sb[:],
                                           trimask[:], op0=Alu.is_equal,
                                           op1=Alu.mult,
                                           accum_out=cnts[:, 0:1])
            mneg = pool.tile([SH, BS], F32)
            nc.vector.scalar_tensor_tensor(mneg[:], trimask[:], 1.0, mpos[:],
                                           op0=Alu.mult, op1=Alu.subtract,
                                           accum_out=cnts[:, 1:2])
            spos = pool.tile([SH, BS], PDT)
            nc.vector.scalar_tensor_tensor(spos[:], mpos[:], 1.0, splus[:],
                                           op0=Alu.mult, op1=Alu.mult)
            sneg = pool.tile([SH, BS], PDT)
            nc.vector.scalar_tensor_tensor(sneg[:], mneg[:], 1.0, splus[:],
                                           op0=Alu.mult, op1=Alu.mult)
            src = {"pos": spos, "neg": sneg}

            # sums of masked s' (for closed-form low bins)
            trashD = pool.tile([SH, BS], PDT)
            trashA = pool.tile([SH, BS], PDT)
            nc.vector.tensor_scalar(trashD[:], spos[:], 1.0, 0.0, op0=Alu.mult,
                                    op1=Alu.add, accum_out=cnts[:, 2:3])
            nc.vector.tensor_scalar(trashA[:], sneg[:], 1.0, 0.0, op0=Alu.mult,
                                    op1=Alu.add, accum_out=cnts[:, 3:4])

            zeros = pool.tile([SH, BS], PDT)
            nc.vector.memset(zeros[:], 0.0)
            accD = pool.tile([SH, max(nD, 1)], F32)
            accG = pool.tile([SH, max(nG, 1)], F32)
            trashG = pool.tile([SH, BS], PDT)
            accA = pool.tile([SH, max(nA, 1)], F32)

            for fam, k in passes:
                eng, j = plan[(fam, k)]
                c = float(np.float32(k * BW))
                s_t = src[fam]
                if eng == "D":
                    nc.vector.scalar_tensor_tensor(trashD[:], s_t[:], c,
                                                   zeros[:], op0=Alu.subtract,
                                                   op1=Alu.max,
                                                   accum_out=accD[:, j:j + 1])
                elif eng == "G":
                    nc.gpsimd.scalar_tensor_tensor(trashG[:], s_t[:], c,
                                                   zeros[:], op0=Alu.subtract,
                                                   op1=Alu.max,
                                                   accum_out=accG[:, j:j + 1])
                else:
                    jc = k - KLO
                    nc.scalar.activation(trashA[:], s_t[:], Act.Relu,
                                         bias=cvec_sb[:, jc:jc + 1], scale=1.0,
                                         accum_out=accA[:, j:j + 1])

            nc.sync.dma_start(acc_out[:, 0:nD], accD[:])
            nc.sync.dma_start(acc_out[:, nD:nD + nA], accA[:])
            if nG:
                nc.sync.dma_start(acc_out[:, nD + nA:nD + nA + nG], accG[:])
            nc.sync.dma_start(acc_out[:, nD + nA + nG:NOUT], cnts[:])

    nc.compile()
    return nc, plan, (nD, nA, nG, NOUT)


def _get_program():
    key = (USE_BAND, KLO, KHI, PASS_DT)
    if key not in _CACHE:
        _CACHE[key] = _build_program()
    return _CACHE[key]


def _host_prep(x, labels):
    x = np.ascontiguousarray(np.asarray(x, dtype=np.float32))
    labels = np.asarray(labels).astype(np.int64)
    perm = np.argsort(labels, kind="stable")
    xs = x[perm]
    labs = labels[perm].astype(np.float32)
    xT = np.ascontiguousarray(xs.T)  # [128, 1024]
    import ml_dtypes
    in_maps = []
    for c in range(N_CORES):
        r = SH * c
        xTr = np.roll(xT, -r, axis=1)
        collab_c = np.ascontiguousarray(np.roll(labs, -r)[None, :])
        rowlab_c = np.ascontiguousarray(collab_c[0, :SH, None])
        wrapcut_c = np.full((SH, 1), float(BS - r), np.float32)
        ks_arr = np.arange(KLO, KHI + 1, dtype=np.float32)
        cvec_c = np.tile(-(ks_arr * np.float32(BW))[None, :], (SH, 1)).astype(np.float32)
        in_maps.append({
            "cvec": cvec_c,
            "xTrL": np.ascontiguousarray(xTr[:, 0:512]),
            "xTrR": np.ascontiguousarray(xTr[:, 512:]),
            "collab": collab_c,
            "collab_bf": collab_c.astype(ml_dtypes.bfloat16),
            "rowlab": rowlab_c,
            "wrapcut": wrapcut_c,
        })
    return in_maps, labels


def _combine(results, plan, meta, labels):
    nD, nA, nG, NOUT = meta
    tot = np.zeros((NOUT,), np.float64)
    gmax = np.full((NOUT,), -np.inf)
    for res in results:
        a = res["acc"].astype(np.float64)
        tot += a.sum(axis=0)
        gmax = np.maximum(gmax, a.max(axis=0))

    def col(eng, j):
        return {"D": 0, "A": nD, "G": nD + nA}[eng] + j

    base = nD + nA + nG
    cntpos = tot[base + 0]
    cntneg = tot[base + 1]
    Spos = tot[base + 2]
    Sneg = tot[base + 3]
    npairs = BS * (BS - 1) // 2
    assert abs(cntpos + cntneg - npairs) < 0.5, (cntpos, cntneg)
    # range guards (zero device cost): R[KLO] must match the closed form
    # (no real value below KLO*BW) and R[KHI] must be ~0 (none above).
    def Rdev(fam, k):
        eng, j = plan[(fam, k)]
        return tot[col(eng, j)]
    ok = True
    for fam, Sm, Nm in (("pos", Spos, cntpos), ("neg", Sneg, cntneg)):
        ok &= abs(Rdev(fam, KLO) - (Sm - Nm * KLO * BW)) < 0.5
        ok &= Rdev(fam, KHI) < 0.5
    if not ok:
        return None  # out-of-range: caller falls back to exact host path

    def R_of(fam, Sm, Nm):
        R = np.zeros((NBINS + 1,), np.float64)  # k = 0..51
        for k in range(NBINS + 1):
            if k < KLO:
                R[k] = Sm - Nm * (k * BW)
            elif k > KHI:
                R[k] = 0.0
            else:
                eng, j = plan[(fam, k)]
                R[k] = tot[col(eng, j)]
        return R

    Rpos = R_of("pos", Spos, cntpos)
    Rneg = R_of("neg", Sneg, cntneg)
    Fpos = (Rpos[:-1] - Rpos[1:]) / BW          # k = 0..50
    Fneg = (Rneg[:-1] - Rneg[1:]) / BW
    Fneg_m1 = cntneg
    histneg = np.empty((NBINS,), np.float64)
    histneg[0] = (Fneg_m1 - Fneg[0]) / cntneg
    histneg[1:] = (Fneg[:-1] - Fneg[1:]) / cntneg
    cdfpos = 1.0 - Fpos / cntpos
    loss = float(np.sum(histneg * cdfpos))
    return np.float32(loss)


def _host_exact(x, labels):
    # exact fallback, only used if the data violates the compiled bin range
    x = np.asarray(x, np.float64)
    labels = np.asarray(labels)
    sim = x @ x.T
    iu, ju = np.triu_indices(x.shape[0], k=1)
    s = sim[iu, ju]
    pos = labels[iu] == labels[ju]
    b = np.floor((s + 1.0) / BW).astype(np.int64)
    v = b * BW - 1.0
    w_lo = (v + BW - s) / BW
    w_hi = (s - v) / BW
    b_hi = np.clip(b + 1, 0, NBINS - 1)

    def hist(m):
        h = np.zeros(NBINS)
        np.add.at(h, b[m], w_lo[m])
        np.add.at(h, b_hi[m], w_hi[m])
        return h / m.sum()

    hp, hn = hist(pos), hist(~pos)
    return np.float32(np.sum(hn * np.cumsum(hp)))


def _run(x, labels, trace=False, trace_cores=None):
    from concourse.bass_utils import run_bass_kernel_spmd
    nc, plan, meta = _get_program()
    in_maps, labels = _host_prep(x, labels)
    out = run_bass_kernel_spmd(nc, in_maps, list(range(N_CORES)),
                               trace=trace, trace_cores=trace_cores)
    loss = _combine(out.results, plan, meta, labels)
    if loss is None:
        loss = _host_exact(x, labels)
    return loss, out


def kernel(x, labels):
    loss, _ = _run(x, labels)
    return loss



# revision 14
# speedup vs baseline: 2.6314x; 2.6314x over previous
"""Trainium2 Bass kernel for the pairwise-similarity histogram loss.

Reference computation:
  sim = x @ x.T  (rows L2-normalized), upper-tri pairs (i<j)
  soft (triangular) binning of similarities into 51 bins, separately for
  label-equal (pos) and label-unequal (neg) pairs; loss = sum(hist_neg * cumsum(hist_pos)).

Device algorithm (8 NeuronCores, SPMD, data-parallel over sim rows):
  Host sorts rows by label and hands each core a column-rotated copy of
  x_sorted.T so every core's own 128 rows sit at rotated columns 0..127 —
  all cores run an identical program.

  Per core, everything is folded into ONE working tile:
    s'' = sim + 4*mposU + 6*trimask - 5        (fp16, [128, 1024])
  where
    - sim + 4*mposU comes straight out of the PE: the contraction is
      augmented with one-hot label columns scaled by 2 (U2 @ U2.T = 4*mposU),
      accumulated into the same PSUM bank as x @ x.T (bf16 matmuls).
    - 6*trimask - 5 is iota-derived (no sim dependency, overlapped with DMA)
      and added by the single PSUM-evacuating scalar_tensor_tensor.
  Then valid neg pairs have s'' = 1+sim in [0,2], valid pos pairs s'' in
  [4,6], and every invalid element is <= 0 — killed by the relu inside
  every histogram pass, so no further masking is needed.

  Histogramming uses the identity
      F[k] = sum_p clip((s_p - k*bw)/bw, 0, 1) = (R[k] - R[k+1]) / bw,
      R[c]  = sum_p relu(s_p - c)
  Each R[c] is ONE instruction: DVE tensor_scalar (sub, max) in fp16 hits
  the 4x DVE fast mode (327ns/pass); the rest go to ACT (Relu + accum).
  "full" passes read the whole [128,1024] tile at c = k*bw (pos pairs
  contribute an exactly-linear term removed on the host); "band" passes
  read a [128,64] diagonal band (label-sorted pos pairs sit within 63
  columns of the diagonal) extracted via a skewed DRAM bounce, at
  c = 4 + k*bw.
  Host: f64 combine of per-partition partials -> final scalar loss.
"""

import numpy as np

NBINS = 51
BW = 2.0 / (NBINS - 1)
BS, D = 1024, 128
N_CORES = 8
SH = BS // N_CORES  # 128 rows per core
N_CLASSES = 32

KLO, KHI = 16, 35          # R[k] computed on device for k in [KLO, KHI]
POS_OFF = 4.0              # pos-pair offset baked into s''
BANDW = 64                 # band covers within-class pair distance 1..64

_CACHE = {}


def _pass_list():
    """(fam, k) passes; k == -1 is the M(base)/S pass, k == -2 the plain-sum
    Stot pass (always on DVE, (mult,add))."""
    ks = [-2, -1] + list(range(KLO, KHI + 1))
    return [("full", k) for k in ks] + [("band", k) for k in ks]


def _pass_c(fam, k):
    base = 0.0 if fam == "full" else POS_OFF
    return float(np.float32(base + (k * BW if k >= 0 else 0.0)))


def _plan_passes():
    """Makespan-optimal engine split with measured per-pass costs.

    DVE passes are tensor_scalar (min, add): accum = M(c) = sum min(s2, c)
    (tensor_scalar's accumulator reduces with op1, so op1 must be add);
    host recovers R(c) = Stot - M(c).  ACT passes are Relu activations with
    bias = -c: accum = R(c) directly.  Stot passes (k == -2) are
    (mult, add) sums pinned to DVE."""
    cost = {("D", "full"): 422.0, ("D", "band"): 173.0,
            ("A", "full"): 1259.0, ("A", "band"): 459.0}
    fulls = [("full", k) for k in [-1] + list(range(KLO, KHI + 1))]
    bands = [("band", k) for k in [-1] + list(range(KLO, KHI + 1))]
    T0, BAND_READY = 6500.0, 10300.0  # pass-phase start, band-tile ready
    best = None
    for fA in range(len(fulls) + 1):
        for bA in range(len(bands) + 1):
            tD = T0 + (len(fulls) - fA + 1) * cost[("D", "full")]
            tD = max(tD, BAND_READY) + (len(bands) - bA + 1) * cost[("D", "band")]
            tA = T0 + fA * cost[("A", "full")]
            tA = max(tA, BAND_READY) + bA * cost[("A", "band")]
            m = max(tD, tA)
            if best is None or m < best[0]:
                best = (m, fA, bA)
    _, fA, bA = best
    plan = {}
    counts = {"D": 0, "A": 0}
    # D: Stot + fulls first (band data arrives late), then its bands
    for p in ([("full", -2)] + fulls[fA:] + [("band", -2)] + bands[bA:]):
        plan[p] = ("D", counts["D"]); counts["D"] += 1
    for p in fulls[:fA] + bands[:bA]:
        plan[p] = ("A", counts["A"]); counts["A"] += 1
    return plan, counts


def _build_program():
    import concourse.bass as bass
    import concourse.bacc as bacc
    import concourse.tile as tile
    import concourse.mybir as mybir

    F32 = mybir.dt.float32
    F16 = mybir.dt.float16
    BF16 = mybir.dt.bfloat16
    Alu = mybir.AluOpType
    Act = mybir.ActivationFunctionType

    plan, counts = _plan_passes()
    nD, nA = counts["D"], counts["A"]
    NOUT = nD + nA

    nc = bacc.Bacc("TRN2", target_bir_lowering=False, debug=False,
                   num_devices=N_CORES)

    # consts: col 0 = wrapcut, cols 1..nA = -c bias values for ACT passes
    consts = nc.dram_tensor("consts", [SH, 1 + nA], F32, kind="ExternalInput")
    xbfA = nc.dram_tensor("xbfA", [D, 512], BF16, kind="ExternalInput")
    xbfB = nc.dram_tensor("xbfB", [D, 512], BF16, kind="ExternalInput")
    u2 = nc.dram_tensor("u2", [N_CLASSES, BS], BF16, kind="ExternalInput")
    acc_out = nc.dram_tensor("acc", [SH, NOUT], F32, kind="ExternalOutput")

    with tile.TileContext(nc) as tc:
        with tc.tile_pool(name="main", bufs=1) as pool, \
             tc.tile_pool(name="psum", bufs=1, space="PSUM") as psum, \
             tc.tile_pool(name="dram", bufs=1, space="DRAM") as dpool:
            xA = pool.tile([D, 512], BF16)
            nc.sync.dma_start(xA[:], xbfA[:])
            consts_sb = pool.tile([SH, 1 + nA], F32)
            nc.sync.dma_start(consts_sb[:], consts[:])
            u2T = pool.tile([N_CLASSES, BS], BF16)
            nc.sync.dma_start(u2T[:], u2[:])
            xB = pool.tile([D, 512], BF16)
            nc.sync.dma_start(xB[:], xbfB[:])

            # iota along free dim (0..1023) fp16, and local row index (0..127)
            iotaT = pool.tile([SH, BS], F16)
            nc.gpsimd.iota(iotaT[:], pattern=[[1, BS]], base=0,
                           channel_multiplier=0,
                           allow_small_or_imprecise_dtypes=True)
            rowloc = pool.tile([SH, 1], F32)
            nc.gpsimd.iota(rowloc[:], pattern=[[0, 1]], base=0,
                           channel_multiplier=1,
                           allow_small_or_imprecise_dtypes=True)

            # Btri6 = 6 * [rowloc < t < wrapcut]   (fp16, no sim dependency)
            gt6 = pool.tile([SH, BS], F16)
            nc.vector.tensor_scalar(gt6[:], iotaT[:], rowloc[:], 6.0,
                                    op0=Alu.is_gt, op1=Alu.mult)
            btri6 = pool.tile([SH, BS], F16)
            nc.vector.scalar_tensor_tensor(btri6[:], iotaT[:],
                                           consts_sb[:, 0:1], gt6[:],
                                           op0=Alu.is_lt, op1=Alu.mult)

            # PSUM: sim + 4*mposU  (bf16 matmuls, U accumulated on top);
            # split PSUM tiles so s2's first half only waits on the A group
            simPA = psum.tile([SH, 512], F32)
            simPB = psum.tile([SH, 512], F32)
            nc.tensor.matmul(simPA[:], xA[:, 0:D], xA[:],
                             start=True, stop=False)
            nc.tensor.matmul(simPA[:], u2T[:, 0:D], u2T[:, 0:512],
                             start=False, stop=True)
            nc.tensor.matmul(simPB[:], xA[:, 0:D], xB[:],
                             start=True, stop=False)
            nc.tensor.matmul(simPB[:], u2T[:, 0:D], u2T[:, 512:BS],
                             start=False, stop=True)

            # s'' = (simP - 5) + btri6   (fp16, PSUM evacuate, two halves)
            s2 = pool.tile([SH, BS], F16)
            nc.vector.scalar_tensor_tensor(s2[:, 0:512], simPA[:], -5.0,
                                           btri6[:, 0:512],
                                           op0=Alu.add, op1=Alu.add)
            nc.vector.scalar_tensor_tensor(s2[:, 512:BS], simPB[:], -5.0,
                                           btri6[:, 512:BS],
                                           op0=Alu.add, op1=Alu.add)

            # diagonal-band bounce: band[p, w] = s2[p, p+1+w], w in [0, 64).
            # Write s2[:, 0:256] (band spans cols <= 191) with row pitch 255:
            # element (p, q) lands at 255p + q, so the band is the CONTIGUOUS
            # read [[256,128],[1,64]] at offset 1. The 1-element row overlap
            # only touches local offsets 0/255, which the band never reads.
            bounce = dpool.tile([SH, 256], F16)
            wap = bounce[:].copy()
            wap.ap = mybir.VecI64Pair([[255, SH], [1, 256]])
            wap.offset = 0
            nc.sync.dma_start(wap, s2[:, 0:256])
            band = pool.tile([SH, BANDW], F16)
            rap = bounce[:].copy()
            rap.ap = mybir.VecI64Pair([[256, SH], [1, BANDW]])
            rap.offset = 1
            nc.sync.dma_start(band[:], rap)

            accD = pool.tile([SH, NOUT], F32)   # cols nD: copied from accA
            accA = pool.tile([SH, max(nA, 1)], F32)
            trashD = pool.tile([SH, BS], F16)
            trashA = pool.tile([SH, BS], F16)
            src = {"full": s2, "band": band}

            for eng in ("D", "A"):
                for (fam, k), (e, j) in sorted(plan.items(), key=lambda kv: kv[1][1]):
                    if e != eng:
                        continue
                    c = _pass_c(fam, k)
                    s_t = src[fam]
                    w = BS if fam == "full" else BANDW
                    if eng == "D":
                        if k == -2:   # Stot: plain sum (accum reduces via op1)
                            nc.vector.tensor_scalar(trashD[:, 0:w], s_t[:], 1.0,
                                                    0.0, op0=Alu.mult,
                                                    op1=Alu.add,
                                                    accum_out=accD[:, j:j + 1])
                        else:         # M(c) = sum min(s2, c); R = Stot - M
                            nc.vector.tensor_scalar(trashD[:, 0:w], s_t[:], c,
                                                    0.0, op0=Alu.min,
                                                    op1=Alu.add,
                                                    accum_out=accD[:, j:j + 1])
                    else:
                        nc.scalar.activation(trashA[:, 0:w], s_t[:], Act.Relu,
                                             bias=consts_sb[:, 1 + j:2 + j],
                                             scale=1.0,
                                             accum_out=accA[:, j:j + 1])

            # fold accA into accD's tail columns -> single output DMA
            nc.vector.tensor_scalar(accD[:, nD:NOUT], accA[:], 1.0, 0.0,
                                    op0=Alu.mult, op1=Alu.add)
            nc.sync.dma_start(acc_out[:], accD[:])

    nc.compile()
    return nc, plan, (nD, nA, NOUT)


def _get_program():
    key = (KLO, KHI)
    if key not in _CACHE:
        _CACHE[key] = _build_program()
    return _CACHE[key]


def _host_prep(x, labels):
    import ml_dtypes
    x = np.ascontiguousarray(np.asarray(x, dtype=np.float32))
    labels = np.asarray(labels).astype(np.int64)
    perm = np.argsort(labels, kind="stable")
    xs = x[perm]
    labs = labels[perm]
    xT = np.ascontiguousarray(xs.T).astype(ml_dtypes.bfloat16)  # [128, 1024]
    # one-hot label rows scaled by 2: U2 @ U2.T = 4 * [lab_i == lab_j]
    U2 = np.zeros((N_CLASSES, BS), np.float32)
    U2[labs, np.arange(BS)] = 2.0
    U2 = U2.astype(ml_dtypes.bfloat16)

    plan, counts = _plan_passes()
    nA = counts["A"]
    acts = sorted(((j, (fam, k)) for (fam, k), (eng, j) in plan.items()
                   if eng == "A"))
    in_maps = []
    for c in range(N_CORES):
        r = SH * c
        consts_c = np.zeros((SH, 1 + nA), np.float32)
        consts_c[:, 0] = float(BS - r)  # wrapcut
        for j, (fam, k) in acts:
            consts_c[:, 1 + j] = -_pass_c(fam, k)
        xTr = np.roll(xT, -r, axis=1)
        in_maps.append({
            "consts": consts_c,
            "xbfA": np.ascontiguousarray(xTr[:, 0:512]),
            "xbfB": np.ascontiguousarray(xTr[:, 512:]),
            "u2": np.ascontiguousarray(np.roll(U2, -r, axis=1)),
        })
    return in_maps, labels


def _combine(results, plan, meta, labels):
    nD, nA, NOUT = meta
    tot = np.zeros((NOUT,), np.float64)
    for res in results:
        tot += res["acc"].astype(np.float64).sum(axis=0)

    def raw(fam, k):
        eng, j = plan[(fam, k)]
        return tot[(0 if eng == "D" else nD) + j], eng

    Stot = {fam: raw(fam, -2)[0] for fam in ("full", "band")}

    def Rdev(fam, k):
        v, eng = raw(fam, k)
        if eng == "A":
            return v            # ACT: R(c) directly (Relu sum)
        return Stot[fam] - v    # DVE: M(c); R(c) = Stot - M(c)

    labs = np.asarray(labels)
    cnts = np.bincount(labs, minlength=1)
    if cnts.max() > BANDW + 1:
        return None  # a class exceeds the band: fall back to exact host path
    Npos = float(sum(int(c) * (int(c) - 1) // 2 for c in cnts))
    npairs = BS * (BS - 1) // 2
    Nneg = float(npairs) - Npos

    Spos = Rdev("band", -1)
    Rall0 = Rdev("full", -1)
    Sneg = Rall0 - Spos - POS_OFF * Npos

    def R_of(fam, Sm, Nm):
        R = np.zeros((NBINS + 1,), np.float64)
        for k in range(NBINS + 1):
            if k < KLO:
                R[k] = Sm - Nm * (k * BW)
            elif k > KHI:
                R[k] = 0.0
            else:
                R[k] = Rdev(fam, k)
                if fam == "full":  # remove the exactly-linear pos contribution
                    R[k] -= Spos + (POS_OFF - k * BW) * Npos
        return R

    Rpos = R_of("band", Spos, Npos)
    Rneg = R_of("full", Sneg, Nneg)
    # range guards: closed form must match at KLO, and R[KHI] ~ 0
    ok = True
    ok &= abs(Rpos[KLO] - (Spos - Npos * KLO * BW)) < 5.0
    ok &= abs(Rneg[KLO] - (Sneg - Nneg * KLO * BW)) < 10.0
    ok &= Rpos[KHI] < 1.0
    ok &= Rneg[KHI] < 2.0
    if not ok:
        return None  # out-of-range: caller falls back to exact host path

    Fpos = (Rpos[:-1] - Rpos[1:]) / BW          # k = 0..50
    Fneg = (Rneg[:-1] - Rneg[1:]) / BW
    histneg = np.empty((NBINS,), np.float64)
    histneg[0] = (Nneg - Fneg[0]) / Nneg
    histneg[1:] = (Fneg[:-1] - Fneg[1:]) / Nneg
    cdfpos = 1.0 - Fpos / Npos
    loss = float(np.sum(histneg * cdfpos))
    return np.float32(loss)


def _host_exact(x, labels):
    # exact fallback, only used if the data violates the compiled bin range
    x = np.asarray(x, np.float64)
    labels = np.asarray(labels)
    sim = x @ x.T
    iu, ju = np.triu_indices(x.shape[0], k=1)
    s = sim[iu, ju]
    pos = labels[iu] == labels[ju]
    b = np.floor((s + 1.0) / BW).astype(np.int64)
    v = b * BW - 1.0
    w_lo = (v + BW - s) / BW
    w_hi = (s - v) / BW
    b_hi = np.clip(b + 1, 0, NBINS - 1)

    def hist(m):
        h = np.zeros(NBINS)
        np.add.at(h, b[m], w_lo[m])
        np.add.at(h, b_hi[m], w_hi[m])
        return h / m.sum()

    hp, hn = hist(pos), hist(~pos)
    return np.float32(np.sum(hn * np.cumsum(hp)))


def _run(x, labels, trace=False, trace_cores=None):
    from concourse.bass_utils import run_bass_kernel_spmd
    nc, plan, meta = _get_program()
    in_maps, labels = _host_prep(x, labels)
    out = run_bass_kernel_spmd(nc, in_maps, list(range(N_CORES)),
                               trace=trace, trace_cores=trace_cores)
    loss = _combine(out.results, plan, meta, labels)
    if loss is None:
        loss = _host_exact(x, labels)
    return loss, out


def kernel(x, labels):
    loss, _ = _run(x, labels)
    return loss


# revision 24
# speedup vs baseline: 2.7868x; 1.0591x over previous
"""Trainium2 Bass kernel for the pairwise-similarity histogram loss.

Reference computation:
  sim = x @ x.T  (rows L2-normalized), upper-tri pairs (i<j)
  soft (triangular) binning of similarities into 51 bins, separately for
  label-equal (pos) and label-unequal (neg) pairs; loss = sum(hist_neg * cumsum(hist_pos)).

Device algorithm (8 NeuronCores, SPMD, data-parallel over sim rows):
  Host sorts rows by label and hands each core a column-rotated copy of
  x_sorted.T so every core's own 128 rows sit at rotated columns 0..127 —
  all cores run an identical program.

  Per core, everything is folded into ONE working tile:
    s'' = sim + 4*mposU + 6*trimask - 5        (fp16, [128, 1024])
  where
    - sim + 4*mposU comes straight out of the PE: the contraction is
      augmented with one-hot label columns scaled by 2 (U2 @ U2.T = 4*mposU),
      accumulated into the same PSUM bank as x @ x.T (bf16 matmuls).
    - 6*trimask - 5 is iota-derived (no sim dependency, overlapped with DMA)
      and added by the single PSUM-evacuating scalar_tensor_tensor.
  Then valid neg pairs have s'' = 1+sim in [0,2], valid pos pairs s'' in
  [4,6], and every invalid element is <= 0 — killed by the relu inside
  every histogram pass, so no further masking is needed.

  Histogramming uses the identity
      F[k] = sum_p clip((s_p - k*bw)/bw, 0, 1) = (R[k] - R[k+1]) / bw,
      R[c]  = sum_p relu(s_p - c)
  Each R[c] is ONE instruction: DVE tensor_scalar (sub, max) in fp16 hits
  the 4x DVE fast mode (327ns/pass); the rest go to ACT (Relu + accum).
  "full" passes read the whole [128,1024] tile at c = k*bw (pos pairs
  contribute an exactly-linear term removed on the host); "band" passes
  read a [128,64] diagonal band (label-sorted pos pairs sit within 63
  columns of the diagonal) extracted via a skewed DRAM bounce, at
  c = 4 + k*bw.
  Host: f64 combine of per-partition partials -> final scalar loss.
"""

import numpy as np

NBINS = 51
BW = 2.0 / (NBINS - 1)
BS, D = 1024, 128
N_CORES = 8
SH = BS // N_CORES  # 128 rows per core
N_CLASSES = 32

KLO, KHI = 17, 34          # R[k] computed on device for k in [KLO, KHI]
POS_OFF = 4.0              # pos-pair offset baked into s''
BANDW = 64                 # band covers within-class pair distance 1..64

_CACHE = {}


def _pass_list():
    """(fam, k) passes; k == -1 is the M(base)/S pass, k == -2 the plain-sum
    Stot pass (always on DVE, (mult,add))."""
    ks = [-2, -1] + list(range(KLO, KHI + 1))
    return [("full", k) for k in ks] + [("band", k) for k in ks]


def _pass_c(fam, k):
    base = 0.0 if fam == "full" else POS_OFF
    return float(np.float32(base + (k * BW if k >= 0 else 0.0)))


def _plan_passes():
    """Makespan-optimal engine split with measured per-pass costs.

    DVE passes are tensor_scalar (min, add): accum = M(c) = sum min(s2, c)
    (tensor_scalar's accumulator reduces with op1, so op1 must be add);
    host recovers R(c) = Stot - M(c).  ACT passes are Relu activations with
    bias = -c: accum = R(c) directly.  Stot passes (k == -2) are
    (mult, add) sums pinned to DVE."""
    cost = {("D", "full"): 422.0, ("D", "band"): 173.0,
            ("A", "full"): 1259.0, ("A", "band"): 459.0}
    fulls = [("full", k) for k in [-1] + list(range(KLO, KHI + 1))]
    bands = [("band", k) for k in [-1] + list(range(KLO, KHI + 1))]
    # pass-phase starts; ACT additionally evacuates s2's B half (~600ns)
    T0, A0, BAND_READY = 6100.0, 6500.0, 10200.0
    best = None
    for fA in range(len(fulls) + 1):
        for bA in range(len(bands) + 1):
            tD = T0 + (len(fulls) - fA + 1) * cost[("D", "full")]
            tD = max(tD, BAND_READY) + (len(bands) - bA + 1) * cost[("D", "band")]
            tA = A0 + fA * cost[("A", "full")]
            tA = max(tA, BAND_READY) + bA * cost[("A", "band")]
            m = max(tD, tA)
            if best is None or m < best[0]:
                best = (m, fA, bA)
    _, fA, bA = best
    plan = {}
    counts = {"D": 0, "A": 0}
    # D: Stot + fulls first (band data arrives late), then its bands.
    # A takes fulls from the TAIL so D keeps the early ks it half-splits.
    nf = len(fulls)
    d_passes = [("full", -2)] + fulls[:nf - fA] + [("band", -2)] + bands[bA:]
    # first 3 D full passes are emitted as two half-tile halves (2 cols each)
    half = set(p for p in d_passes if p[0] == "full")
    half = set(list(sorted(half, key=lambda p: p[1]))[:3])
    for p in d_passes:
        plan[p] = ("D", counts["D"])
        counts["D"] += 2 if p in half else 1
    for p in fulls[nf - fA:] + bands[:bA]:
        plan[p] = ("A", counts["A"]); counts["A"] += 1
    return plan, counts, half


def _build_program():
    import concourse.bass as bass
    import concourse.bacc as bacc
    import concourse.tile as tile
    import concourse.mybir as mybir

    F32 = mybir.dt.float32
    F16 = mybir.dt.float16
    BF16 = mybir.dt.bfloat16
    Alu = mybir.AluOpType
    Act = mybir.ActivationFunctionType

    plan, counts, half = _plan_passes()
    nD, nA = counts["D"], counts["A"]
    NOUT = nD + nA

    nc = bacc.Bacc("TRN2", target_bir_lowering=False, debug=False,
                   num_devices=N_CORES)

    # consts: col 0 = wrapcut, cols 1..nA = -c bias values for ACT passes
    consts = nc.dram_tensor("consts", [SH, 1 + nA], F32, kind="ExternalInput")
    xbfA = nc.dram_tensor("xbfA", [D, 512], BF16, kind="ExternalInput")
    xbfB = nc.dram_tensor("xbfB", [D, 512], BF16, kind="ExternalInput")
    # u2all: rows 0..31 = one-hot labels scaled by 2 (mov cols 0..1023,
    # stat cols 1024..1151); row 32 = B-half mask row: stat 1, mov
    # 6*[t<wrap]-5 on cols 512..1023 (0 on the A half, where the full
    # btri6 tensor is added instead).
    u2all = nc.dram_tensor("u2all", [N_CLASSES + 1, BS + D], BF16,
                           kind="ExternalInput")
    acc_out = nc.dram_tensor("acc", [SH, NOUT], F32, kind="ExternalOutput")

    with tile.TileContext(nc) as tc:
        with tc.tile_pool(name="main", bufs=1) as pool, \
             tc.tile_pool(name="psum", bufs=1, space="PSUM") as psum, \
             tc.tile_pool(name="dram", bufs=1, space="DRAM") as dpool:
            xA = pool.tile([D, 512], BF16)
            nc.sync.dma_start(xA[:], xbfA[:])
            consts_sb = pool.tile([SH, 1 + nA], F32)
            nc.sync.dma_start(consts_sb[:], consts[:])
            u2T = pool.tile([N_CLASSES + 1, BS + D], BF16)
            nc.sync.dma_start(u2T[:], u2all[:])
            xB = pool.tile([D, 512], BF16)
            nc.sync.dma_start(xB[:], xbfB[:])

            # iota along free dim (0..511) fp16, and local row index (0..127)
            iotaT = pool.tile([SH, 512], F16)
            nc.gpsimd.iota(iotaT[:], pattern=[[1, 512]], base=0,
                           channel_multiplier=0,
                           allow_small_or_imprecise_dtypes=True)
            rowloc = pool.tile([SH, 1], F32)
            nc.gpsimd.iota(rowloc[:], pattern=[[0, 1]], base=0,
                           channel_multiplier=1,
                           allow_small_or_imprecise_dtypes=True)

            # Btri6 = 6 * [rowloc < t < wrapcut], A half only (t < 512; the
            # B half's mask is partition-independent and rides the matmul)
            gt6 = pool.tile([SH, 512], F16)
            nc.vector.tensor_scalar(gt6[:], iotaT[:], rowloc[:], 6.0,
                                    op0=Alu.is_gt, op1=Alu.mult)
            btri6 = pool.tile([SH, 512], F16)
            nc.vector.scalar_tensor_tensor(btri6[:], iotaT[:],
                                           consts_sb[:, 0:1], gt6[:],
                                           op0=Alu.is_lt, op1=Alu.mult)

            # PSUM A: sim + 4*mposU; PSUM B: sim + 4*mposU + 6*[t<wrap] - 5
            # (33-row contraction; bf16 matmuls, U accumulated on top)
            simPA = psum.tile([SH, 512], F32)
            simPB = psum.tile([SH, 512], F32)
            US, UM = BS, 0   # stat cols base, mov cols base in u2T
            nc.tensor.matmul(simPA[:], xA[:, 0:D], xA[:],
                             start=True, stop=False)
            nc.tensor.matmul(simPA[:], u2T[:, US:US + D], u2T[:, UM:UM + 512],
                             start=False, stop=True)
            nc.tensor.matmul(simPB[:], xA[:, 0:D], xB[:],
                             start=True, stop=False)
            nc.tensor.matmul(simPB[:], u2T[:, US:US + D],
                             u2T[:, UM + 512:UM + BS],
                             start=False, stop=True)

            # s'' halves: A = (simPA - 5) + btri6 on DVE; B = simPB copy on ACT
            s2 = pool.tile([SH, BS], F16)
            nc.vector.scalar_tensor_tensor(s2[:, 0:512], simPA[:], -5.0,
                                           btri6[:],
                                           op0=Alu.add, op1=Alu.add)
            nc.scalar.activation(s2[:, 512:BS], simPB[:], Act.Identity,
                                 bias=0.0, scale=1.0)

            # diagonal-band bounce: band[p, w] = s2[p, p+1+w], w in [0, 64).
            # Write s2[:, 0:256] (band spans cols <= 191) with row pitch 255:
            # element (p, q) lands at 255p + q, so the band is the CONTIGUOUS
            # read [[256,128],[1,64]] at offset 1. The 1-element row overlap
            # only touches local offsets 0/255, which the band never reads.
            bounce = dpool.tile([SH, 256], F16)
            wap = bounce[:].copy()
            wap.ap = mybir.VecI64Pair([[255, SH], [1, 256]])
            wap.offset = 0
            nc.sync.dma_start(wap, s2[:, 0:256])
            band = pool.tile([SH, BANDW], F16)
            rap = bounce[:].copy()
            rap.ap = mybir.VecI64Pair([[256, SH], [1, BANDW]])
            rap.offset = 1
            nc.sync.dma_start(band[:], rap)

            accD = pool.tile([SH, NOUT], F32)   # cols nD: copied from accA
            accA = pool.tile([SH, max(nA, 1)], F32)
            trashD = pool.tile([SH, BS], F16)
            trashA = pool.tile([SH, BS], F16)
            src = {"full": s2, "band": band}

            def d_pass(in_ap, w, k, c, col):
                if k == -2:   # Stot: plain sum (accum reduces via op1)
                    nc.vector.tensor_scalar(trashD[:, 0:w], in_ap, 1.0, 0.0,
                                            op0=Alu.mult, op1=Alu.add,
                                            accum_out=accD[:, col:col + 1])
                else:         # M(c) = sum min(s2, c); R = Stot - M
                    nc.vector.tensor_scalar(trashD[:, 0:w], in_ap, c, 0.0,
                                            op0=Alu.min, op1=Alu.add,
                                            accum_out=accD[:, col:col + 1])

            d_order = sorted(((j, p) for p, (e, j) in plan.items() if e == "D"))
            # h1 halves right after s2's A half (fills DVE while ACT writes B)
            for j, (fam, k) in d_order:
                if (fam, k) in half:
                    d_pass(s2[:, 0:512], 512, k, _pass_c(fam, k), j)
            for j, (fam, k) in d_order:
                c = _pass_c(fam, k)
                if (fam, k) in half:
                    d_pass(s2[:, 512:BS], 512, k, c, j + 1)
                elif fam == "full":
                    d_pass(s2[:], BS, k, c, j)
                else:
                    d_pass(band[:], BANDW, k, c, j)
            for (fam, k), (e, j) in sorted(plan.items(), key=lambda kv: kv[1][1]):
                if e != "A":
                    continue
                w = BS if fam == "full" else BANDW
                nc.scalar.activation(trashA[:, 0:w], src[fam][:], Act.Relu,
                                     bias=consts_sb[:, 1 + j:2 + j],
                                     scale=1.0,
                                     accum_out=accA[:, j:j + 1])

            # fold accA into accD's tail columns -> single output DMA
            nc.vector.tensor_scalar(accD[:, nD:NOUT], accA[:], 1.0, 0.0,
                                    op0=Alu.mult, op1=Alu.add)
            nc.sync.dma_start(acc_out[:], accD[:])

    nc.compile()
    return nc, plan, (nD, nA, NOUT, half)


def _get_program():
    key = (KLO, KHI)
    if key not in _CACHE:
        _CACHE[key] = _build_program()
    return _CACHE[key]


def _host_prep(x, labels):
    import ml_dtypes
    x = np.ascontiguousarray(np.asarray(x, dtype=np.float32))
    labels = np.asarray(labels).astype(np.int64)
    perm = np.argsort(labels, kind="stable")
    xs = x[perm]
    labs = labels[perm]
    xT = np.ascontiguousarray(xs.T).astype(ml_dtypes.bfloat16)  # [128, 1024]
    # one-hot label rows scaled by 2: U2 @ U2.T = 4 * [lab_i == lab_j]
    U2 = np.zeros((N_CLASSES, BS), np.float32)
    U2[labs, np.arange(BS)] = 2.0

    plan, counts, half = _plan_passes()
    nA = counts["A"]
    acts = sorted(((j, (fam, k)) for (fam, k), (eng, j) in plan.items()
                   if eng == "A"))
    in_maps = []
    for c in range(N_CORES):
        r = SH * c
        consts_c = np.zeros((SH, 1 + nA), np.float32)
        consts_c[:, 0] = float(BS - r)  # wrapcut
        for j, (fam, k) in acts:
            consts_c[:, 1 + j] = -_pass_c(fam, k)
        xTr = np.roll(xT, -r, axis=1)
        # u2all: rows 0..31 one-hot (mov | stat); row 32 = B-half mask row
        u2c = np.zeros((N_CLASSES + 1, BS + D), np.float32)
        u2c[0:N_CLASSES, 0:BS] = np.roll(U2, -r, axis=1)
        u2c[0:N_CLASSES, BS:BS + D] = u2c[0:N_CLASSES, 0:D]
        t = np.arange(512, BS)
        u2c[N_CLASSES, 512:BS] = 6.0 * (t < (BS - r)) - 5.0
        u2c[N_CLASSES, BS:BS + D] = 1.0
        in_maps.append({
            "consts": consts_c,
            "xbfA": np.ascontiguousarray(xTr[:, 0:512]),
            "xbfB": np.ascontiguousarray(xTr[:, 512:]),
            "u2all": u2c.astype(ml_dtypes.bfloat16),
        })
    return in_maps, labels


def _combine(results, plan, meta, labels):
    nD, nA, NOUT, half = meta
    tot = np.zeros((NOUT,), np.float64)
    for res in results:
        tot += res["acc"].astype(np.float64).sum(axis=0)

    def raw(fam, k):
        eng, j = plan[(fam, k)]
        if eng == "D":
            v = tot[j]
            if (fam, k) in half:
                v += tot[j + 1]
            return v, eng
        return tot[nD + j], eng

    Stot = {fam: raw(fam, -2)[0] for fam in ("full", "band")}

    def Rdev(fam, k):
        v, eng = raw(fam, k)
        if eng == "A":
            return v            # ACT: R(c) directly (Relu sum)
        return Stot[fam] - v    # DVE: M(c); R(c) = Stot - M(c)

    labs = np.asarray(labels)
    cnts = np.bincount(labs, minlength=1)
    if cnts.max() > BANDW + 1:
        return None  # a class exceeds the band: fall back to exact host path
    Npos = float(sum(int(c) * (int(c) - 1) // 2 for c in cnts))
    npairs = BS * (BS - 1) // 2
    Nneg = float(npairs) - Npos

    Spos = Rdev("band", -1)
    Rall0 = Rdev("full", -1)
    Sneg = Rall0 - Spos - POS_OFF * Npos

    def R_of(fam, Sm, Nm):
        R = np.zeros((NBINS + 1,), np.float64)
        for k in range(NBINS + 1):
            if k < KLO:
                R[k] = Sm - Nm * (k * BW)
            elif k > KHI:
                R[k] = 0.0
            else:
                R[k] = Rdev(fam, k)
                if fam == "full":  # remove the exactly-linear pos contribution
                    R[k] -= Spos + (POS_OFF - k * BW) * Npos
        return R

    Rpos = R_of("band", Spos, Npos)
    Rneg = R_of("full", Sneg, Nneg)
    # range guards: closed form must match at KLO, and R[KHI] ~ 0
    ok = True
    ok &= abs(Rpos[KLO] - (Spos - Npos * KLO * BW)) < 5.0
    ok &= abs(Rneg[KLO] - (Sneg - Nneg * KLO * BW)) < 10.0
    ok &= Rpos[KHI] < 1.0
    ok &= Rneg[KHI] < 2.0
    if not ok:
        return None  # out-of-range: caller falls back to exact host path

    Fpos = (Rpos[:-1] - Rpos[1:]) / BW          # k = 0..50
    Fneg = (Rneg[:-1] - Rneg[1:]) / BW
    histneg = np.empty((NBINS,), np.float64)
    histneg[0] = (Nneg - Fneg[0]) / Nneg
    histneg[1:] = (Fneg[:-1] - Fneg[1:]) / Nneg
    cdfpos = 1.0 - Fpos / Npos
    loss = float(np.sum(histneg * cdfpos))
    return np.float32(loss)


def _host_exact(x, labels):
    # exact fallback, only used if the data violates the compiled bin range
    x = np.asarray(x, np.float64)
    labels = np.asarray(labels)
    sim = x @ x.T
    iu, ju = np.triu_indices(x.shape[0], k=1)
    s = sim[iu, ju]
    pos = labels[iu] == labels[ju]
    b = np.floor((s + 1.0) / BW).astype(np.int64)
    v = b * BW - 1.0
    w_lo = (v + BW - s) / BW
    w_hi = (s - v) / BW
    b_hi = np.clip(b + 1, 0, NBINS - 1)

    def hist(m):
        h = np.zeros(NBINS)
        np.add.at(h, b[m], w_lo[m])
        np.add.at(h, b_hi[m], w_hi[m])
        return h / m.sum()

    hp, hn = hist(pos), hist(~pos)
    return np.float32(np.sum(hn * np.cumsum(hp)))


def _run(x, labels, trace=False, trace_cores=None):
    from concourse.bass_utils import run_bass_kernel_spmd
    nc, plan, meta = _get_program()
    in_maps, labels = _host_prep(x, labels)
    out = run_bass_kernel_spmd(nc, in_maps, list(range(N_CORES)),
                               trace=trace, trace_cores=trace_cores)
    loss = _combine(out.results, plan, meta, labels)
    if loss is None:
        loss = _host_exact(x, labels)
    return loss, out


def kernel(x, labels):
    loss, _ = _run(x, labels)
    return loss
